# revision 5
# baseline (speedup 1.0000x reference)
"""Multi-head attention (b=16, l=1025, d=768, H=12) on 8 TRN2 NeuronCores.

Sharding: data-parallel over batch - 2 batch elements per core, no
collectives.

Per-core kernel v3. Key ideas vs the v1 baseline:

1. Scores via fp8 DoubleRow matmuls: K is split into fp8 hi+lo planes
   (error-compensated), Q is single fp8 (one-sided). One DoubleRow matmul
   computes Khi^T.Qhi + Klo^T.Qhi = K^T.Qhi at 0.5 cycles/row - half the
   bf16 cost. The K bias is dropped entirely (softmax over keys is
   invariant to a per-query constant); the Q bias is folded into Qhi.
2. PV in O-orientation: out O[q, 65] per (head, q-block, key-block) with
   lhsT = P^T slice [128 keys, 128 q], rhs = V_aug [128 keys, 64+1] (a
   ones column per head makes output col 64 the softmax denominator).
   Full contraction + full output partitions; free dim 65 vs the old
   O^T orientation's full-q - PV cost halves and needs no separate
   denominator pass.
3. O is transposed back to O^T for the output projection with the DMA
   xbar transpose ([128,128] chunks) - zero PE/ACT/DVE cost - issued
   incrementally per head-pair so the output projection can chase the
   attention loop.
4. The l=1025 stragglers: query 1024 is fully handled on the host (from
   exported K/V); key 1024's rank-12 contribution to Y is added on the
   host from exported p8 = exp(s8/8) and denominators, where s8 is
   computed on device as [q,1]-oriented N=1 matmuls.

The schedule interleaves projection matmuls (element e+1's QKV, element
e's output projection) into the ACT-paced attention loop as PE filler.
"""

import contextlib

import numpy as np
import ml_dtypes

import concourse.bass as bass
import concourse.bacc as bacc
import concourse.mybir as mybir
import concourse.tile as tile
from concourse.bass_utils import run_bass_kernel_spmd

N_CORES = 8
B = 16
L = 1025
D = 768
H = 12
DH = 64
BPC = B // N_CORES
KT = D // 128   # 6 contraction tiles
NJ = 8          # full 128-key blocks; key 1024 handled via s8/p8
SCALE = 1.0 / np.sqrt(DH)
KLO = 1040      # Klo plane offset inside khilo tiles (16-aligned)
VW = DH + 1     # 65: V_aug block per head (ones column last)

BF16 = mybir.dt.bfloat16
F32 = mybir.dt.float32
FP8 = mybir.dt.float8e4
NPF8 = ml_dtypes.float8_e4m3
EXP = mybir.ActivationFunctionType.Exp
MULT = mybir.AluOpType.mult
ADD = mybir.AluOpType.add
SUB = mybir.AluOpType.subtract
DR = mybir.MatmulPerfMode.DoubleRow

_CACHE = {}


def _ap(t, poff, pcount, foff, fdims):
    """AP on tile t at partition offset poff (count pcount), free offset
    foff with free dims [(step, count), ...]."""
    base = t[:]
    pstep = base.ap[0][0]
    return bass.AP(tensor=base.tensor,
                   offset=base.offset + poff * pstep + foff,
                   ap=[[pstep, pcount]] + [list(d) for d in fdims])


def _dram3(dram_ap, psize, nt, tstride, inner):
    """3D dram AP: [[row, psize], [tstride, nt], [1, inner]]."""
    return bass.AP(tensor=dram_ap.tensor, offset=dram_ap.offset,
                   ap=[[inner, psize], [tstride, nt], [1, inner]])


def _build():
    nc = bacc.Bacc("TRN2", target_bir_lowering=False, debug=False,
                   num_devices=N_CORES)
    xT = nc.dram_tensor("xT", [BPC, D, L], BF16, kind="ExternalInput")
    w_qk = nc.dram_tensor("w_qk", [D, 2 * D], BF16, kind="ExternalInput")
    w_v = nc.dram_tensor("w_v", [D, D], BF16, kind="ExternalInput")
    w_o = nc.dram_tensor("w_o", [D, D], BF16, kind="ExternalInput")
    b_q = nc.dram_tensor("b_q", [D, 1], F32, kind="ExternalInput")
    b_v = nc.dram_tensor("b_v", [1, D], F32, kind="ExternalInput")
    b_o = nc.dram_tensor("b_o", [D, 1], F32, kind="ExternalInput")
    yT = nc.dram_tensor("yT", [BPC, KT, 128, 1024], BF16,
                        kind="ExternalOutput")
    kTo = nc.dram_tensor("kTo", [BPC, KT, 128, 2080], FP8,
                         kind="ExternalOutput")
    vo = nc.dram_tensor("vo", [BPC, 9, 128, H * VW], BF16,
                        kind="ExternalOutput")
    p8o = nc.dram_tensor("p8o", [BPC, 128, 96], BF16, kind="ExternalOutput")
    dno = nc.dram_tensor("dno", [BPC, 128, 96], F32, kind="ExternalOutput")

    with tile.TileContext(nc) as tc:
        _emit(nc, tc, xT, w_qk, w_v, w_o, b_q, b_v, b_o, yT, kTo, vo, p8o,
              dno)
    nc.compile()
    return nc


def _emit(nc, tc, xT, w_qk, w_v, w_o, b_q, b_v, b_o, yT, kTo, vo, p8o, dno):
    ctx = contextlib.ExitStack()
    with ctx:
        consts = ctx.enter_context(tc.tile_pool(name="consts", bufs=1))
        xpool = ctx.enter_context(tc.tile_pool(name="xpool", bufs=2))
        qpool = ctx.enter_context(tc.tile_pool(name="qpool", bufs=2))
        kpool = ctx.enter_context(tc.tile_pool(name="kpool", bufs=2))
        vpool = ctx.enter_context(tc.tile_pool(name="vpool", bufs=2))
        ptpool = ctx.enter_context(tc.tile_pool(name="ptpool", bufs=11))
        osbpool = ctx.enter_context(tc.tile_pool(name="osbpool", bufs=2))
        otpool = ctx.enter_context(tc.tile_pool(name="otpool", bufs=1))
        recpool = ctx.enter_context(tc.tile_pool(name="recpool", bufs=3))
        p8pool = ctx.enter_context(tc.tile_pool(name="p8pool", bufs=2))
        ytpool = ctx.enter_context(tc.tile_pool(name="ytpool", bufs=2))
        # PSUM: scores 2x[128,1024]=4 banks, pv 2x[128,260]=2, proj 2x=2
        bigp = ctx.enter_context(tc.tile_pool(name="bigp", bufs=2,
                                              space="PSUM"))
        pvp = ctx.enter_context(tc.tile_pool(name="pvp", bufs=2,
                                             space="PSUM"))
        projp = ctx.enter_context(tc.tile_pool(name="projp", bufs=2,
                                               space="PSUM"))

        wqk_t = consts.tile([128, KT * 2 * D], BF16, name="wqk")
        wv_t = consts.tile([128, KT * D], BF16, name="wv")
        wo_t = consts.tile([128, KT * D], BF16, name="wo")
        bq_t = consts.tile([128, KT], F32, name="bq")
        bo_t = consts.tile([128, KT], F32, name="bo")
        bv_bc = consts.tile([128, D], F32, name="bvbc")

        def wqk(k, a, n):
            return _ap(wqk_t, 0, 128, k * 2 * D + a, [[1, n]])

        def wv(k, a, n):
            return _ap(wv_t, 0, 128, k * D + a, [[1, n]])

        def wo(k, a, n):
            return _ap(wo_t, 0, 128, k * D + a, [[1, n]])

        xt = {}
        qhi = {}
        khilo = {}
        vt = {}
        osb = {}
        oTt = {}
        p8 = {}
        dnF = {}

        def alloc_elem(e):
            qhi[e] = [qpool.tile([128, 1024], FP8, tag=f"qhi{m}",
                                 name=f"qhi{e}_{m}") for m in range(KT)]
            khilo[e] = [kpool.tile([128, 2080], FP8, tag=f"kh{m}",
                                   name=f"kh{e}_{m}") for m in range(KT)]
            vt[e] = [vpool.tile([128, H * VW], BF16, tag=f"vt{j}",
                                name=f"vt{e}_{j}") for j in range(9)]
            osb[e] = osbpool.tile([128, 8 * D], BF16, tag="osb",
                                  name=f"osb{e}")
            oTt[e] = otpool.tile([128, KT * 1024], BF16, tag="oT",
                                 name=f"oT{e}")
            p8[e] = p8pool.tile([128, 96], BF16, tag="p8", name=f"p8_{e}")
            dnF[e] = p8pool.tile([128, 96], F32, tag="dn", name=f"dn{e}")

        def xtap(e, k, a, n):
            return _ap(xt[e], 0, 128, k * L + a, [[1, n]])

        def load_x(e):
            xt[e] = xpool.tile([128, KT * L], BF16, tag="xt", name=f"xt{e}")
            nc.sync.dma_start(
                out=_ap(xt[e], 0, 128, 0, [[L, KT], [1, L]]),
                in_=_dram3(xT[e], 128, KT, 128 * L, L))

        def load_x_gen(e):
            load_x(e)
            yield

        def v_unit(e, j):
            """V_aug tile [jlen, 12*65]: per head 64 V cols + ones col."""
            jlen = min(128, L - j * 128)
            nc.vector.memset(
                _ap(vt[e][j], 0, 128, DH, [[VW, H], [1, 1]]), 1.0)
            for (c0, nh) in ((0, 8), (512, 4)):
                w = nh * DH
                ps = projp.tile([128, 512], F32, tag="proj",
                                name=f"vps{e}_{j}_{c0}")
                for k in range(KT):
                    nc.tensor.matmul(ps[:jlen, :w],
                                     xtap(e, k, j * 128, jlen),
                                     wv(k, c0, w),
                                     start=(k == 0), stop=(k == KT - 1))
                    if k % 2 == 1:
                        yield
                nc.vector.tensor_tensor(
                    out=_ap(vt[e][j], 0, jlen, (c0 // DH) * VW,
                            [[VW, nh], [1, DH]]),
                    in0=_ap(ps, 0, jlen, 0, [[DH, nh], [1, DH]]),
                    in1=_ap(bv_bc, 0, jlen, c0, [[DH, nh], [1, DH]]),
                    op=ADD)
            nc.sync.dma_start(out=vo[e, j][0:jlen, :], in_=vt[e][j][:jlen, :])

        def qk_unit(e, m):
            """m 0..5: Q m-tile -> qhi (fp8, +bias). m 6..11: K m-tile ->
            khilo hi/lo planes (fp8, biasless) + straggler col + export."""
            for c in (0, 1):
                ps = projp.tile([128, 512], F32, tag="proj",
                                name=f"qkps{e}_{m}_{c}")
                for k in range(KT):
                    nc.tensor.matmul(ps[:, :],
                                     wqk(k, m * 128, 128),
                                     xtap(e, k, c * 512, 512),
                                     start=(k == 0), stop=(k == KT - 1))
                    if k % 2 == 1:
                        yield
                if m < KT:
                    nc.vector.tensor_scalar_add(
                        out=_ap(qhi[e][m], 0, 128, c * 512, [[1, 512]]),
                        in0=ps[:, :],
                        scalar1=_ap(bq_t, 0, 128, m, [[1, 1]]))
                else:
                    kh = khilo[e][m - KT]
                    nc.vector.tensor_copy(
                        out=_ap(kh, 0, 128, c * 512, [[1, 512]]),
                        in_=ps[:, :])
                    nc.vector.tensor_tensor(
                        out=_ap(kh, 0, 128, KLO + c * 512, [[1, 512]]),
                        in0=ps[:, :],
                        in1=_ap(kh, 0, 128, c * 512, [[1, 512]]), op=SUB)
                    yield
            if m >= KT:
                kh = khilo[e][m - KT]
                ps = projp.tile([128, 512], F32, tag="proj",
                                name=f"qksg{e}_{m}")
                for k in range(KT):
                    nc.tensor.matmul(ps[:, 0:1],
                                     wqk(k, m * 128, 128),
                                     xtap(e, k, 1024, 1),
                                     start=(k == 0), stop=(k == KT - 1))
                yield
                nc.vector.tensor_copy(out=_ap(kh, 0, 128, 1024, [[1, 1]]),
                                      in_=ps[:, 0:1])
                nc.vector.tensor_tensor(
                    out=_ap(kh, 0, 128, KLO + 1024, [[1, 1]]),
                    in0=ps[:, 0:1], in1=_ap(kh, 0, 128, 1024, [[1, 1]]),
                    op=SUB)
                nc.sync.dma_start(out=kTo[e, m - KT], in_=kh[:])

        def o_unit(e, m):
            yt = ytpool.tile([128, 1024], BF16, tag="yt", name=f"yt{e}_{m}")
            for c in (0, 1):
                ps = projp.tile([128, 512], F32, tag="proj",
                                name=f"ops{e}_{m}_{c}")
                for k in range(KT):
                    nc.tensor.matmul(
                        ps[:, :], wo(k, m * 128, 128),
                        _ap(oTt[e], 0, 128, k * 1024 + c * 512, [[1, 512]]),
                        start=(k == 0), stop=(k == KT - 1))
                    if k % 2 == 1:
                        yield
                nc.vector.tensor_scalar_add(
                    out=yt[:, c * 512:c * 512 + 512], in0=ps[:, :],
                    scalar1=_ap(bo_t, 0, 128, m, [[1, 1]]))
            nc.sync.dma_start(out=yT[e, m], in_=yt[:])

        class Fill:
            def __init__(self, gens):
                self.gens = list(gens)

            def pull(self, n=1):
                while n > 0 and self.gens:
                    try:
                        next(self.gens[0])
                        n -= 1
                    except StopIteration:
                        self.gens.pop(0)

            def finish(self, k):
                for gen in self.gens[:k]:
                    for _ in gen:
                        pass
                self.gens = self.gens[k:]

            def append(self, gen):
                self.gens.append(gen)

            def flush(self):
                self.finish(len(self.gens))

        def attn_head(e, h, fill):
            mq, poff = h // 2, (h % 2) * 64
            kh, qh = khilo[e][mq], qhi[e][mq]
            # straggler-key scores s8[q, 1] (N=1 matmuls via a proj chunk),
            # then p8 = exp(s8/8) for the host-side rank-12 correction
            ps8 = projp.tile([128, 512], F32, tag="proj", name=f"s8_{e}_{h}")
            for qb in range(8):
                nc.tensor.matmul(_ap(ps8, 0, 128, qb, [[1, 1]]),
                                 _ap(qh, poff, 64, qb * 128, [[1, 128]]),
                                 _ap(kh, poff, 64, 1024, [[1, 1]]),
                                 start=True, stop=True)
            nc.scalar.activation(_ap(p8[e], 0, 128, h * 8, [[1, 8]]),
                                 _ap(ps8, 0, 128, 0, [[1, 8]]),
                                 EXP, bias=0.0, scale=float(SCALE))
            # scores: one DoubleRow matmul per (j, q-half):
            # S^T = Khi^T.Qhi + Klo^T.Qhi
            pts = []
            for j in range(NJ):
                sps = bigp.tile([128, 1024], F32, tag="big",
                                name=f"sps{e}_{h}_{j}")
                for c in (0, 1):
                    nc.tensor.matmul(
                        sps[:, c * 512:c * 512 + 512],
                        _ap(kh, poff, 64, j * 128, [[KLO, 2], [1, 128]]),
                        _ap(qh, poff, 64, c * 512, [[0, 2], [1, 512]]),
                        start=True, stop=True, perf_mode=DR)
                pt = ptpool.tile([128, 1024], BF16, tag="pt",
                                 name=f"pt{e}_{h}_{j}")
                nc.scalar.activation(pt[:, :], sps[:, :], EXP,
                                     bias=0.0, scale=float(SCALE))
                pts.append(pt)
                fill.pull(2)
            # PV (O-orientation, V_aug): per (q-half c, q-block qb) one
            # psum group of 8 consecutive j-matmuls; out [128 q, 65] with
            # col 64 = softmax denominator
            for c in (0, 1):
                pv = pvp.tile([128, 260], F32, tag="pv",
                              name=f"pv{e}_{h}_{c}")
                for qb in range(4):
                    qg = c * 4 + qb
                    for j in range(NJ):
                        nc.tensor.matmul(
                            pv[:, qb * VW:qb * VW + VW],
                            pts[j][:, qg * 128:qg * 128 + 128],
                            vt[e][j][:, h * VW:h * VW + VW],
                            start=(j == 0), stop=(j == NJ - 1))
                    fill.pull(1)
                # normalize: rec = 1/(D8 + p8); O_sb = pv[:, :64] * rec
                nc.vector.tensor_tensor(
                    out=_ap(dnF[e], 0, 128, h * 8 + c * 4, [[1, 4]]),
                    in0=_ap(pv, 0, 128, DH, [[VW, 4]]),
                    in1=_ap(p8[e], 0, 128, h * 8 + c * 4, [[1, 4]]),
                    op=ADD)
                rec = recpool.tile([128, 4], F32, tag="rec",
                                   name=f"rec{e}_{h}_{c}")
                nc.vector.reciprocal(rec[:, :],
                                     _ap(dnF[e], 0, 128, h * 8 + c * 4,
                                         [[1, 4]]))
                nc.vector.tensor_tensor(
                    out=_ap(osb[e], 0, 128, c * 4 * D + h * 64,
                            [[D, 4], [1, 64]]),
                    in0=_ap(pv, 0, 128, 0, [[VW, 4], [1, 64]]),
                    in1=_ap(rec, 0, 128, 0, [[1, 4], [0, 64]]), op=MULT)
                fill.pull(1)

        def pair_transpose(e, m):
            """xbar-transpose the finished head-pair m (cols m*128..) of
            O_sb into O^T rows m*128.., for all 8 q-blocks."""
            for qb in range(8):
                nc.sync.dma_start(
                    out=_ap(oTt[e], 0, 128, m * 1024 + qb * 128, [[1, 128]]),
                    in_=_ap(osb[e], 0, 128, qb * D + m * 128, [[1, 128]]),
                    transpose=True)

        # ---- schedule ----
        # warm the exp table during the input DMA shadow
        warm = recpool.tile([1, 4], F32, tag="warm", name="warm")
        nc.vector.memset(warm[:1, 0:1], 0.0)
        nc.scalar.activation(warm[:1, 0:1], warm[:1, 0:1], EXP,
                             bias=0.0, scale=1.0)
        load_x(0)
        nc.sync.dma_start(out=_ap(wqk_t, 0, 128, 0, [[2 * D, KT], [1, 2 * D]]),
                          in_=_dram3(w_qk[:, :], 128, KT, 128 * 2 * D, 2 * D))
        nc.sync.dma_start(out=_ap(wv_t, 0, 128, 0, [[D, KT], [1, D]]),
                          in_=_dram3(w_v[:, :], 128, KT, 128 * D, D))
        bva = b_v[:]
        nc.sync.dma_start(out=bv_bc[:], in_=bass.AP(
            tensor=bva.tensor, offset=bva.offset,
            ap=[[0, 128], list(bva.ap[1])]))
        nc.sync.dma_start(out=_ap(bq_t, 0, 128, 0, [[1, KT]]),
                          in_=_dram3(b_q[:, :], 128, KT, 128, 1))
        nc.sync.dma_start(out=_ap(bo_t, 0, 128, 0, [[1, KT]]),
                          in_=_dram3(b_o[:, :], 128, KT, 128, 1))
        nc.sync.dma_start(out=_ap(wo_t, 0, 128, 0, [[D, KT], [1, D]]),
                          in_=_dram3(w_o[:, :], 128, KT, 128 * D, D))

        def run(gen):
            for _ in gen:
                pass

        alloc_elem(0)
        run(qk_unit(0, 0))
        run(qk_unit(0, 6))
        for j in range(9):
            run(v_unit(0, j))

        gens = []
        for m in range(1, KT):
            gens += [qk_unit(0, m), qk_unit(0, KT + m)]
        gens += [load_x_gen(1)]
        alloc_elem(1)
        gens += [v_unit(1, j) for j in range(9)]
        for m in range(KT):
            gens += [qk_unit(1, m), qk_unit(1, KT + m)]
        fill = Fill(gens)
        for h in range(H):
            if h >= 2 and h % 2 == 0:
                fill.finish(2)
            attn_head(0, h, fill)
            if h % 2 == 1:
                # transpose the finished head-pair (DMA xbar, off-engine)
                pair_transpose(0, h // 2)
        nc.sync.dma_start(out=p8o[0], in_=p8[0][:])
        nc.sync.dma_start(out=dno[0], in_=dnF[0][:])
        fill.flush()
        # element 0's output projection (needs ALL of O^T(0)) fills
        # element 1's attention; delay its first pull one head so the
        # last transposes have landed before its matmuls hit the PE queue
        fill2 = Fill([])
        for h in range(H):
            attn_head(1, h, fill2)
            if h == 0:
                for m in range(KT):
                    fill2.append(o_unit(0, m))
            if h % 2 == 1:
                pair_transpose(1, h // 2)
        nc.sync.dma_start(out=p8o[1], in_=p8[1][:])
        nc.sync.dma_start(out=dno[1], in_=dnF[1][:])
        fill2.flush()
        for m in range(KT):
            run(o_unit(1, m))


def _prep_inputs(query, Wqkv, bqkv, Wo, bo):
    Wp = Wqkv.reshape(D, 3, DH, H).transpose(0, 1, 3, 2).reshape(D, 3 * D)
    bp = bqkv.reshape(3, DH, H).transpose(0, 2, 1).reshape(3 * D)
    w_qk = np.ascontiguousarray(Wp[:, :2 * D]).astype(ml_dtypes.bfloat16)
    w_v = np.ascontiguousarray(Wp[:, 2 * D:]).astype(ml_dtypes.bfloat16)
    w_o = np.ascontiguousarray(Wo).astype(ml_dtypes.bfloat16)
    b_q = np.ascontiguousarray(bp[:D]).astype(np.float32).reshape(D, 1)
    b_v = np.ascontiguousarray(bp[2 * D:]).astype(np.float32).reshape(1, D)
    b_o = np.ascontiguousarray(bo).astype(np.float32).reshape(D, 1)

    in_maps = []
    for c in range(N_CORES):
        xc = query[c * BPC:(c + 1) * BPC]
        xTc = np.ascontiguousarray(xc.transpose(0, 2, 1)).astype(
            ml_dtypes.bfloat16)
        in_maps.append(dict(xT=xTc, w_qk=w_qk, w_v=w_v, w_o=w_o,
                            b_q=b_q, b_v=b_v, b_o=b_o))
    return in_maps


def kernel(query, Wqkv, bqkv, Wo, bo):
    query = np.asarray(query, dtype=np.float32)
    Wqkv = np.asarray(Wqkv, dtype=np.float32)
    bqkv = np.asarray(bqkv, dtype=np.float32)
    Wo = np.asarray(Wo, dtype=np.float32)
    bo = np.asarray(bo, dtype=np.float32)

    if "nc" not in _CACHE:
        _CACHE["nc"] = _build()
    nc = _CACHE["nc"]

    in_maps = _prep_inputs(query, Wqkv, bqkv, Wo, bo)
    res = run_bass_kernel_spmd(nc, in_maps, core_ids=list(range(N_CORES)))

    Wp = Wqkv.reshape(D, 3, DH, H).transpose(0, 1, 3, 2).reshape(D, 3 * D)
    bp = bqkv.reshape(3, DH, H).transpose(0, 2, 1).reshape(3 * D)
    out = np.empty((B, L, D), dtype=np.float32)
    for c in range(N_CORES):
        r = res.results[c]
        for e in range(BPC):
            b = c * BPC + e
            # main output: Y^T tiles [6, 128, 1024] -> Y [1024, 768]
            y = np.asarray(r["yT"][e], dtype=np.float32).reshape(
                D, 1024).T.copy()
            # rank-12 straggler-key correction: Y += (p8/D) @ (v_1024 Wo_h)
            p8v = np.asarray(r["p8o"][e], dtype=np.float32)
            dnv = np.asarray(r["dno"][e], dtype=np.float32)
            p8n = (p8v / dnv).reshape(128, H, 8).transpose(2, 0, 1).reshape(
                1024, H)
            vfull = np.asarray(r["vo"][e], dtype=np.float32).reshape(
                9 * 128, H, VW)[:, :, :DH]
            v1024 = vfull[1024]  # [H, 64]
            w8v = np.einsum("hd,hde->he", v1024, Wo.reshape(H, DH, D))
            y += p8n @ w8v
            out[b, :1024] = y
            # straggler query row: exact host attention from exported K/V
            kt8 = np.asarray(r["kTo"][e], dtype=np.float32)  # [6, 128, 2080]
            kT = (kt8[:, :, :L] + kt8[:, :, KLO:KLO + L]).reshape(D, L)
            qrow = query[b, L - 1] @ Wp[:, :D] + bp[:D]
            orow = np.empty(D, dtype=np.float32)
            for h in range(H):
                kh = kT[h * DH:(h + 1) * DH]  # [64, L]
                sh = (qrow[h * DH:(h + 1) * DH] @ kh) * SCALE
                ph = np.exp(sh - sh.max())
                vh = vfull[:L, h]
                orow[h * DH:(h + 1) * DH] = (ph @ vh) / ph.sum()
            out[b, L - 1] = orow @ Wo + bo
    return out


# revision 11
# speedup vs baseline: 1.1613x; 1.1613x over previous
"""Multi-head attention (b=16, l=1025, d=768, H=12) on 8 TRN2 NeuronCores.

Sharding: data-parallel over batch - 2 batch elements per core, no
collectives.

Per-core kernel v3. Key ideas vs the v1 baseline:

1. Scores via fp8 DoubleRow matmuls: K is split into fp8 hi+lo planes
   (error-compensated), Q is single fp8 (one-sided). One DoubleRow matmul
   computes Khi^T.Qhi + Klo^T.Qhi = K^T.Qhi at 0.5 cycles/row - half the
   bf16 cost. The K bias is dropped entirely (softmax over keys is
   invariant to a per-query constant); the Q bias is folded into Qhi.
2. PV in O-orientation: out O[q, 65] per (head, q-block, key-block) with
   lhsT = P^T slice [128 keys, 128 q], rhs = V_aug [128 keys, 64+1] (a
   ones column per head makes output col 64 the softmax denominator).
   Full contraction + full output partitions; free dim 65 vs the old
   O^T orientation's full-q - PV cost halves and needs no separate
   denominator pass.
3. O is transposed back to O^T for the output projection with the DMA
   xbar transpose ([128,128] chunks) - zero PE/ACT/DVE cost - issued
   incrementally per head-pair so the output projection can chase the
   attention loop.
4. The l=1025 stragglers: query 1024 is fully handled on the host (from
   exported K/V); key 1024's rank-12 contribution to Y is added on the
   host from exported p8 = exp(s8/8) and denominators, where s8 is
   computed on device as [q,1]-oriented N=1 matmuls.

The schedule interleaves projection matmuls (element e+1's QKV, element
e's output projection) into the ACT-paced attention loop as PE filler.
"""

import contextlib

import numpy as np
import ml_dtypes

import concourse.bass as bass
import concourse.bacc as bacc
import concourse.mybir as mybir
import concourse.tile as tile
from concourse.bass_utils import run_bass_kernel_spmd

N_CORES = 8
B = 16
L = 1025
D = 768
H = 12
DH = 64
BPC = B // N_CORES
KT = D // 128   # 6 contraction tiles
NJ = 8          # full 128-key blocks; key 1024 handled via s8/p8
SCALE = 1.0 / np.sqrt(DH)
KLO = 1040      # Klo plane offset inside khilo tiles (16-aligned)
VW = DH + 1     # 65: V_aug block per head (ones column last)

BF16 = mybir.dt.bfloat16
F32 = mybir.dt.float32
FP8 = mybir.dt.float8e4
NPF8 = ml_dtypes.float8_e4m3
EXP = mybir.ActivationFunctionType.Exp
MULT = mybir.AluOpType.mult
ADD = mybir.AluOpType.add
SUB = mybir.AluOpType.subtract
DR = mybir.MatmulPerfMode.DoubleRow

_CACHE = {}


def _ap(t, poff, pcount, foff, fdims):
    """AP on tile t at partition offset poff (count pcount), free offset
    foff with free dims [(step, count), ...]."""
    base = t[:]
    pstep = base.ap[0][0]
    return bass.AP(tensor=base.tensor,
                   offset=base.offset + poff * pstep + foff,
                   ap=[[pstep, pcount]] + [list(d) for d in fdims])


def _dram3(dram_ap, psize, nt, tstride, inner):
    """3D dram AP: [[row, psize], [tstride, nt], [1, inner]]."""
    return bass.AP(tensor=dram_ap.tensor, offset=dram_ap.offset,
                   ap=[[inner, psize], [tstride, nt], [1, inner]])


def _build():
    nc = bacc.Bacc("TRN2", target_bir_lowering=False, debug=False,
                   num_devices=N_CORES)
    xT = nc.dram_tensor("xT", [BPC, D, L], BF16, kind="ExternalInput")
    w_qk = nc.dram_tensor("w_qk", [D, 2 * D], BF16, kind="ExternalInput")
    w_v = nc.dram_tensor("w_v", [D, D], BF16, kind="ExternalInput")
    w_o = nc.dram_tensor("w_o", [D, D], BF16, kind="ExternalInput")
    b_q = nc.dram_tensor("b_q", [D, 1], F32, kind="ExternalInput")
    b_v = nc.dram_tensor("b_v", [1, D], F32, kind="ExternalInput")
    b_o = nc.dram_tensor("b_o", [D, 1], F32, kind="ExternalInput")
    yT = nc.dram_tensor("yT", [BPC, KT, 128, 1024], BF16,
                        kind="ExternalOutput")
    kTo = nc.dram_tensor("kTo", [BPC, KT, 128, 2080], FP8,
                         kind="ExternalOutput")
    vo = nc.dram_tensor("vo", [BPC, 9, 128, H * VW], BF16,
                        kind="ExternalOutput")
    p8o = nc.dram_tensor("p8o", [BPC, 128, 96], BF16, kind="ExternalOutput")
    dno = nc.dram_tensor("dno", [BPC, 128, 96], F32, kind="ExternalOutput")

    with tile.TileContext(nc) as tc:
        _emit(nc, tc, xT, w_qk, w_v, w_o, b_q, b_v, b_o, yT, kTo, vo, p8o,
              dno)
    nc.compile()
    return nc


def _emit(nc, tc, xT, w_qk, w_v, w_o, b_q, b_v, b_o, yT, kTo, vo, p8o, dno):
    ctx = contextlib.ExitStack()
    with ctx:
        consts = ctx.enter_context(tc.tile_pool(name="consts", bufs=1))
        xpool = ctx.enter_context(tc.tile_pool(name="xpool", bufs=2))
        qpool = ctx.enter_context(tc.tile_pool(name="qpool", bufs=2))
        kpool = ctx.enter_context(tc.tile_pool(name="kpool", bufs=2))
        vpool = ctx.enter_context(tc.tile_pool(name="vpool", bufs=2))
        ptpool = ctx.enter_context(tc.tile_pool(name="ptpool", bufs=16))
        osbpool = ctx.enter_context(tc.tile_pool(name="osbpool", bufs=2))
        otpool = ctx.enter_context(tc.tile_pool(name="otpool", bufs=1))
        recpool = ctx.enter_context(tc.tile_pool(name="recpool", bufs=3))
        p8pool = ctx.enter_context(tc.tile_pool(name="p8pool", bufs=2))
        ytpool = ctx.enter_context(tc.tile_pool(name="ytpool", bufs=2))
        # PSUM: scores 2x[128,1024]=4 banks, pv 2x[128,260]=2, proj 2x=2
        bigp = ctx.enter_context(tc.tile_pool(name="bigp", bufs=2,
                                              space="PSUM"))
        pvp = ctx.enter_context(tc.tile_pool(name="pvp", bufs=2,
                                             space="PSUM"))
        projp = ctx.enter_context(tc.tile_pool(name="projp", bufs=2,
                                               space="PSUM"))

        wqk_t = consts.tile([128, KT * 2 * D], BF16, name="wqk")
        wv_t = consts.tile([128, KT * D], BF16, name="wv")
        wo_t = consts.tile([128, KT * D], BF16, name="wo")
        bq_t = consts.tile([128, KT], F32, name="bq")
        bo_t = consts.tile([128, KT], F32, name="bo")
        bv_bc = consts.tile([128, D], F32, name="bvbc")

        def wqk(k, a, n):
            return _ap(wqk_t, 0, 128, k * 2 * D + a, [[1, n]])

        def wv(k, a, n):
            return _ap(wv_t, 0, 128, k * D + a, [[1, n]])

        def wo(k, a, n):
            return _ap(wo_t, 0, 128, k * D + a, [[1, n]])

        xt = {}
        qhi = {}
        khilo = {}
        vt = {}
        osb = {}
        oTt = {}
        p8 = {}
        dnF = {}

        def alloc_elem(e):
            qhi[e] = [qpool.tile([128, 1024], FP8, tag=f"qhi{m}",
                                 name=f"qhi{e}_{m}") for m in range(KT)]
            khilo[e] = [kpool.tile([128, 2080], FP8, tag=f"kh{m}",
                                   name=f"kh{e}_{m}") for m in range(KT)]
            vt[e] = [vpool.tile([128, H * VW], BF16, tag=f"vt{j}",
                                name=f"vt{e}_{j}") for j in range(9)]
            osb[e] = osbpool.tile([128, 8 * D], BF16, tag="osb",
                                  name=f"osb{e}")
            oTt[e] = otpool.tile([128, KT * 1024], BF16, tag="oT",
                                 name=f"oT{e}")
            p8[e] = p8pool.tile([128, 96], BF16, tag="p8", name=f"p8_{e}")
            dnF[e] = p8pool.tile([128, 96], F32, tag="dn", name=f"dn{e}")

        def xtap(e, k, a, n):
            return _ap(xt[e], 0, 128, k * L + a, [[1, n]])

        def load_x(e):
            xt[e] = xpool.tile([128, KT * L], BF16, tag="xt", name=f"xt{e}")
            nc.sync.dma_start(
                out=_ap(xt[e], 0, 128, 0, [[L, KT], [1, L]]),
                in_=_dram3(xT[e], 128, KT, 128 * L, L))

        def load_x_gen(e):
            load_x(e)
            yield

        def v_unit(e, j):
            """V_aug tile [jlen, 12*65]: per head 64 V cols + ones col."""
            jlen = min(128, L - j * 128)
            nc.vector.memset(
                _ap(vt[e][j], 0, 128, DH, [[VW, H], [1, 1]]), 1.0)
            for (c0, nh) in ((0, 8), (512, 4)):
                w = nh * DH
                ps = projp.tile([128, 512], F32, tag="proj",
                                name=f"vps{e}_{j}_{c0}")
                for k in range(KT):
                    nc.tensor.matmul(ps[:jlen, :w],
                                     xtap(e, k, j * 128, jlen),
                                     wv(k, c0, w),
                                     start=(k == 0), stop=(k == KT - 1))
                    if k % 2 == 1:
                        yield
                nc.vector.tensor_tensor(
                    out=_ap(vt[e][j], 0, jlen, (c0 // DH) * VW,
                            [[VW, nh], [1, DH]]),
                    in0=_ap(ps, 0, jlen, 0, [[DH, nh], [1, DH]]),
                    in1=_ap(bv_bc, 0, jlen, c0, [[DH, nh], [1, DH]]),
                    op=ADD)
            nc.sync.dma_start(out=vo[e, j][0:jlen, :], in_=vt[e][j][:jlen, :])

        def qk_unit(e, m):
            """m 0..5: Q m-tile -> qhi (fp8, +bias). m 6..11: K m-tile ->
            khilo hi/lo planes (fp8, biasless) + straggler col + export."""
            for c in (0, 1):
                ps = projp.tile([128, 512], F32, tag="proj",
                                name=f"qkps{e}_{m}_{c}")
                for k in range(KT):
                    nc.tensor.matmul(ps[:, :],
                                     wqk(k, m * 128, 128),
                                     xtap(e, k, c * 512, 512),
                                     start=(k == 0), stop=(k == KT - 1))
                    if k % 2 == 1:
                        yield
                if m < KT:
                    nc.vector.tensor_scalar_add(
                        out=_ap(qhi[e][m], 0, 128, c * 512, [[1, 512]]),
                        in0=ps[:, :],
                        scalar1=_ap(bq_t, 0, 128, m, [[1, 1]]))
                else:
                    kh = khilo[e][m - KT]
                    nc.vector.tensor_copy(
                        out=_ap(kh, 0, 128, c * 512, [[1, 512]]),
                        in_=ps[:, :])
                    nc.vector.tensor_tensor(
                        out=_ap(kh, 0, 128, KLO + c * 512, [[1, 512]]),
                        in0=ps[:, :],
                        in1=_ap(kh, 0, 128, c * 512, [[1, 512]]), op=SUB)
                    yield
            if m >= KT:
                kh = khilo[e][m - KT]
                ps = projp.tile([128, 512], F32, tag="proj",
                                name=f"qksg{e}_{m}")
                for k in range(KT):
                    nc.tensor.matmul(ps[:, 0:1],
                                     wqk(k, m * 128, 128),
                                     xtap(e, k, 1024, 1),
                                     start=(k == 0), stop=(k == KT - 1))
                yield
                nc.vector.tensor_copy(out=_ap(kh, 0, 128, 1024, [[1, 1]]),
                                      in_=ps[:, 0:1])
                nc.vector.tensor_tensor(
                    out=_ap(kh, 0, 128, KLO + 1024, [[1, 1]]),
                    in0=ps[:, 0:1], in1=_ap(kh, 0, 128, 1024, [[1, 1]]),
                    op=SUB)
                nc.sync.dma_start(out=kTo[e, m - KT], in_=kh[:])

        def o_unit(e, m):
            yt = ytpool.tile([128, 1024], BF16, tag="yt", name=f"yt{e}_{m}")
            for c in (0, 1):
                ps = projp.tile([128, 512], F32, tag="proj",
                                name=f"ops{e}_{m}_{c}")
                for k in range(KT):
                    nc.tensor.matmul(
                        ps[:, :], wo(k, m * 128, 128),
                        _ap(oTt[e], 0, 128, k * 1024 + c * 512, [[1, 512]]),
                        start=(k == 0), stop=(k == KT - 1))
                    if k % 2 == 1:
                        yield
                nc.vector.tensor_scalar_add(
                    out=yt[:, c * 512:c * 512 + 512], in0=ps[:, :],
                    scalar1=_ap(bo_t, 0, 128, m, [[1, 1]]))
            nc.sync.dma_start(out=yT[e, m], in_=yt[:])

        class Fill:
            def __init__(self, gens):
                self.gens = list(gens)

            def pull(self, n=1):
                while n > 0 and self.gens:
                    try:
                        next(self.gens[0])
                        n -= 1
                    except StopIteration:
                        self.gens.pop(0)

            def finish(self, k):
                for gen in self.gens[:k]:
                    for _ in gen:
                        pass
                self.gens = self.gens[k:]

            def append(self, gen):
                self.gens.append(gen)

            def flush(self):
                self.finish(len(self.gens))

        # ---- software-pipelined attention: head h's score/exp loop
        # overlaps head h-1's PV accumulation so ACT never idles ----
        state = {"pend": None, "pts": None, "pv": None}

        def pv_step(qg):
            """One q-block-group of the pending head's PV: 8 consecutive
            matmuls accumulating over all key blocks. Groups sharing a
            psum bank must be consecutive - a start=True clears the whole
            bank's has_written bits on hardware."""
            e2, h2 = state["pend"]
            pts = state["pts"]
            pva, pvb = state["pv"]
            pv = pva if qg < 4 else pvb
            qb = qg % 4
            for j in range(NJ):
                nc.tensor.matmul(
                    pv[:, qb * VW:qb * VW + VW],
                    pts[j][:, qg * 128:qg * 128 + 128],
                    vt[e2][j][:, h2 * VW:h2 * VW + VW],
                    start=(j == 0), stop=(j == NJ - 1))

        def finalize_pend():
            """Normalize the pending head: rec = 1/(D8+p8); O_sb = pv*rec."""
            e2, h2 = state["pend"]
            pva, pvb = state["pv"]
            for c, pv in ((0, pva), (1, pvb)):
                nc.vector.tensor_tensor(
                    out=_ap(dnF[e2], 0, 128, h2 * 8 + c * 4, [[1, 4]]),
                    in0=_ap(pv, 0, 128, DH, [[VW, 4]]),
                    in1=_ap(p8[e2], 0, 128, h2 * 8 + c * 4, [[1, 4]]),
                    op=ADD)
                rec = recpool.tile([128, 4], F32, tag="rec",
                                   name=f"rec{e2}_{h2}_{c}")
                nc.vector.reciprocal(rec[:, :],
                                     _ap(dnF[e2], 0, 128, h2 * 8 + c * 4,
                                         [[1, 4]]))
                nc.vector.tensor_tensor(
                    out=_ap(osb[e2], 0, 128, c * 4 * D + h2 * 64,
                            [[D, 4], [1, 64]]),
                    in0=_ap(pv, 0, 128, 0, [[VW, 4], [1, 64]]),
                    in1=_ap(rec, 0, 128, 0, [[1, 4], [0, 64]]), op=MULT)
            if h2 % 2 == 1:
                pair_transpose(e2, h2 // 2)
            if h2 == H - 1:
                nc.sync.dma_start(out=p8o[e2], in_=p8[e2][:])
                nc.sync.dma_start(out=dno[e2], in_=dnF[e2][:])
            state["pend"] = None

        def head_loop(e, h, fill):
            mq, poff = h // 2, (h % 2) * 64
            kh, qh = khilo[e][mq], qhi[e][mq]
            # scores: one DoubleRow matmul per (j, q-half):
            # S^T = Khi^T.Qhi + Klo^T.Qhi; PV of the previous head rides
            # along one j-step at a time
            pts = []
            for j in range(NJ):
                sps = bigp.tile([128, 1024], F32, tag="big",
                                name=f"sps{e}_{h}_{j}")
                for c in (0, 1):
                    nc.tensor.matmul(
                        sps[:, c * 512:c * 512 + 512],
                        _ap(kh, poff, 64, j * 128, [[KLO, 2], [1, 128]]),
                        _ap(qh, poff, 64, c * 512, [[0, 2], [1, 512]]),
                        start=True, stop=True, perf_mode=DR)
                pt = ptpool.tile([128, 1024], BF16, tag="pt",
                                 name=f"pt{e}_{h}_{j}")
                nc.scalar.activation(pt[:, :], sps[:, :], EXP,
                                     bias=0.0, scale=float(SCALE))
                pts.append(pt)
                if state["pend"] is not None:
                    pv_step(j)
                fill.pull(2)
            if state["pend"] is not None:
                finalize_pend()
            # allocate this head's PV accumulators (after the previous
            # head's normalize released the slots); straggler-key scores
            # s8 ride in spare columns of the c0 accumulator bank
            pva = pvp.tile([128, 512], F32, tag="pv", name=f"pv{e}_{h}_0")
            pvb = pvp.tile([128, 512], F32, tag="pv", name=f"pv{e}_{h}_1")
            for qb in range(8):
                nc.tensor.matmul(_ap(pva, 0, 128, 300 + qb, [[1, 1]]),
                                 _ap(qh, poff, 64, qb * 128, [[1, 128]]),
                                 _ap(kh, poff, 64, 1024, [[1, 1]]),
                                 start=True, stop=True)
            nc.scalar.activation(_ap(p8[e], 0, 128, h * 8, [[1, 8]]),
                                 _ap(pva, 0, 128, 300, [[1, 8]]),
                                 EXP, bias=0.0, scale=float(SCALE))
            state["pend"] = (e, h)
            state["pts"] = pts
            state["pv"] = (pva, pvb)
            fill.pull(1)

        def drain_pend(fill):
            for j in range(NJ):
                pv_step(j)
                fill.pull(1)
            finalize_pend()

        def pair_transpose(e, m):
            """xbar-transpose the finished head-pair m (cols m*128..) of
            O_sb into O^T rows m*128.., for all 8 q-blocks."""
            for qb in range(8):
                nc.sync.dma_start(
                    out=_ap(oTt[e], 0, 128, m * 1024 + qb * 128, [[1, 128]]),
                    in_=_ap(osb[e], 0, 128, qb * D + m * 128, [[1, 128]]),
                    transpose=True)

        # ---- schedule ----
        # warm the exp table during the input DMA shadow
        warm = recpool.tile([1, 4], F32, tag="warm", name="warm")
        nc.vector.memset(warm[:1, 0:1], 0.0)
        nc.scalar.activation(warm[:1, 0:1], warm[:1, 0:1], EXP,
                             bias=0.0, scale=1.0)
        load_x(0)
        nc.sync.dma_start(out=_ap(wqk_t, 0, 128, 0, [[2 * D, KT], [1, 2 * D]]),
                          in_=_dram3(w_qk[:, :], 128, KT, 128 * 2 * D, 2 * D))
        nc.sync.dma_start(out=_ap(wv_t, 0, 128, 0, [[D, KT], [1, D]]),
                          in_=_dram3(w_v[:, :], 128, KT, 128 * D, D))
        bva = b_v[:]
        nc.sync.dma_start(out=bv_bc[:], in_=bass.AP(
            tensor=bva.tensor, offset=bva.offset,
            ap=[[0, 128], list(bva.ap[1])]))
        nc.sync.dma_start(out=_ap(bq_t, 0, 128, 0, [[1, KT]]),
                          in_=_dram3(b_q[:, :], 128, KT, 128, 1))
        nc.sync.dma_start(out=_ap(bo_t, 0, 128, 0, [[1, KT]]),
                          in_=_dram3(b_o[:, :], 128, KT, 128, 1))
        nc.sync.dma_start(out=_ap(wo_t, 0, 128, 0, [[D, KT], [1, D]]),
                          in_=_dram3(w_o[:, :], 128, KT, 128 * D, D))

        def run(gen):
            for _ in gen:
                pass

        alloc_elem(0)
        run(qk_unit(0, 0))
        run(qk_unit(0, 6))
        for j in range(9):
            run(v_unit(0, j))

        gens = []
        for m in range(1, KT):
            gens += [qk_unit(0, m), qk_unit(0, KT + m)]
        gens += [load_x_gen(1)]
        alloc_elem(1)
        gens += [v_unit(1, j) for j in range(9)]
        gens += [qk_unit(1, 0), qk_unit(1, KT)]
        fill = Fill(gens)
        for h in range(H):
            if h >= 2 and h % 2 == 0:
                fill.finish(2)
            head_loop(0, h, fill)
        fill.flush()
        # element 1's attention: its scores pipeline over element 0's
        # last PV; element 0's output projection (needs ALL of O^T(0),
        # finalized inside head (1,0)) fills from head (1,1) on
        gens2 = []
        for m in range(1, KT):
            gens2 += [qk_unit(1, m), qk_unit(1, KT + m)]
        fill2 = Fill(gens2)
        for h in range(H):
            if h >= 2 and h % 2 == 0:
                fill2.finish(2)
            head_loop(1, h, fill2)
            if h == 1:
                for m in range(KT):
                    fill2.append(o_unit(0, m))
        drain_pend(fill2)
        fill2.flush()
        for m in range(KT):
            run(o_unit(1, m))


def _prep_inputs(query, Wqkv, bqkv, Wo, bo):
    Wp = Wqkv.reshape(D, 3, DH, H).transpose(0, 1, 3, 2).reshape(D, 3 * D)
    bp = bqkv.reshape(3, DH, H).transpose(0, 2, 1).reshape(3 * D)
    w_qk = np.ascontiguousarray(Wp[:, :2 * D]).astype(ml_dtypes.bfloat16)
    w_v = np.ascontiguousarray(Wp[:, 2 * D:]).astype(ml_dtypes.bfloat16)
    w_o = np.ascontiguousarray(Wo).astype(ml_dtypes.bfloat16)
    b_q = np.ascontiguousarray(bp[:D]).astype(np.float32).reshape(D, 1)
    b_v = np.ascontiguousarray(bp[2 * D:]).astype(np.float32).reshape(1, D)
    b_o = np.ascontiguousarray(bo).astype(np.float32).reshape(D, 1)

    in_maps = []
    for c in range(N_CORES):
        xc = query[c * BPC:(c + 1) * BPC]
        xTc = np.ascontiguousarray(xc.transpose(0, 2, 1)).astype(
            ml_dtypes.bfloat16)
        in_maps.append(dict(xT=xTc, w_qk=w_qk, w_v=w_v, w_o=w_o,
                            b_q=b_q, b_v=b_v, b_o=b_o))
    return in_maps


def kernel(query, Wqkv, bqkv, Wo, bo):
    query = np.asarray(query, dtype=np.float32)
    Wqkv = np.asarray(Wqkv, dtype=np.float32)
    bqkv = np.asarray(bqkv, dtype=np.float32)
    Wo = np.asarray(Wo, dtype=np.float32)
    bo = np.asarray(bo, dtype=np.float32)

    if "nc" not in _CACHE:
        _CACHE["nc"] = _build()
    nc = _CACHE["nc"]

    in_maps = _prep_inputs(query, Wqkv, bqkv, Wo, bo)
    res = run_bass_kernel_spmd(nc, in_maps, core_ids=list(range(N_CORES)))

    Wp = Wqkv.reshape(D, 3, DH, H).transpose(0, 1, 3, 2).reshape(D, 3 * D)
    bp = bqkv.reshape(3, DH, H).transpose(0, 2, 1).reshape(3 * D)
    out = np.empty((B, L, D), dtype=np.float32)
    for c in range(N_CORES):
        r = res.results[c]
        for e in range(BPC):
            b = c * BPC + e
            # main output: Y^T tiles [6, 128, 1024] -> Y [1024, 768]
            y = np.asarray(r["yT"][e], dtype=np.float32).reshape(
                D, 1024).T.copy()
            # rank-12 straggler-key correction: Y += (p8/D) @ (v_1024 Wo_h)
            p8v = np.asarray(r["p8o"][e], dtype=np.float32)
            dnv = np.asarray(r["dno"][e], dtype=np.float32)
            p8n = (p8v / dnv).reshape(128, H, 8).transpose(2, 0, 1).reshape(
                1024, H)
            vfull = np.asarray(r["vo"][e], dtype=np.float32).reshape(
                9 * 128, H, VW)[:, :, :DH]
            v1024 = vfull[1024]  # [H, 64]
            w8v = np.einsum("hd,hde->he", v1024, Wo.reshape(H, DH, D))
            y += p8n @ w8v
            out[b, :1024] = y
            # straggler query row: exact host attention from exported K/V
            kt8 = np.asarray(r["kTo"][e], dtype=np.float32)  # [6, 128, 2080]
            kT = (kt8[:, :, :L] + kt8[:, :, KLO:KLO + L]).reshape(D, L)
            qrow = query[b, L - 1] @ Wp[:, :D] + bp[:D]
            orow = np.empty(D, dtype=np.float32)
            for h in range(H):
                kh = kT[h * DH:(h + 1) * DH]  # [64, L]
                sh = (qrow[h * DH:(h + 1) * DH] @ kh) * SCALE
                ph = np.exp(sh - sh.max())
                vh = vfull[:L, h]
                orow[h * DH:(h + 1) * DH] = (ph @ vh) / ph.sum()
            out[b, L - 1] = orow @ Wo + bo
    return out


# revision 14
# speedup vs baseline: 1.2482x; 1.0749x over previous
"""Multi-head attention (b=16, l=1025, d=768, H=12) on 8 TRN2 NeuronCores.

Sharding: data-parallel over batch - 2 batch elements per core, no
collectives.

Per-core kernel v3. Key ideas vs the v1 baseline:

1. Scores via fp8 DoubleRow matmuls: K is split into fp8 hi+lo planes
   (error-compensated), Q is single fp8 (one-sided). One DoubleRow matmul
   computes Khi^T.Qhi + Klo^T.Qhi = K^T.Qhi at 0.5 cycles/row - half the
   bf16 cost. The K bias is dropped entirely (softmax over keys is
   invariant to a per-query constant); the Q bias is folded into Qhi.
2. PV in O-orientation: out O[q, 65] per (head, q-block, key-block) with
   lhsT = P^T slice [128 keys, 128 q], rhs = V_aug [128 keys, 64+1] (a
   ones column per head makes output col 64 the softmax denominator).
   Full contraction + full output partitions; free dim 65 vs the old
   O^T orientation's full-q - PV cost halves and needs no separate
   denominator pass.
3. O is transposed back to O^T for the output projection with the DMA
   xbar transpose ([128,128] chunks) - zero PE/ACT/DVE cost - issued
   incrementally per head-pair so the output projection can chase the
   attention loop.
4. The l=1025 stragglers: query 1024 is fully handled on the host (from
   exported K/V); key 1024's rank-12 contribution to Y is added on the
   host from exported p8 = exp(s8/8) and denominators, where s8 is
   computed on device as [q,1]-oriented N=1 matmuls.

The schedule interleaves projection matmuls (element e+1's QKV, element
e's output projection) into the ACT-paced attention loop as PE filler.
"""

import contextlib

import numpy as np
import ml_dtypes

import concourse.bass as bass
import concourse.bacc as bacc
import concourse.mybir as mybir
import concourse.tile as tile
from concourse.bass_utils import run_bass_kernel_spmd

N_CORES = 8
B = 16
L = 1025
D = 768
H = 12
DH = 64
BPC = B // N_CORES
KT = D // 128   # 6 contraction tiles
NJ = 8          # full 128-key blocks; key 1024 handled via s8/p8
SCALE = 1.0 / np.sqrt(DH)
KLO = 1040      # Klo plane offset inside khilo tiles (16-aligned)
VW = DH + 1     # 65: V_aug block per head (ones column last)

BF16 = mybir.dt.bfloat16
F32 = mybir.dt.float32
FP8 = mybir.dt.float8e4
NPF8 = ml_dtypes.float8_e4m3
EXP = mybir.ActivationFunctionType.Exp
MULT = mybir.AluOpType.mult
ADD = mybir.AluOpType.add
SUB = mybir.AluOpType.subtract
DR = mybir.MatmulPerfMode.DoubleRow

_CACHE = {}


def _ap(t, poff, pcount, foff, fdims):
    """AP on tile t at partition offset poff (count pcount), free offset
    foff with free dims [(step, count), ...]."""
    base = t[:]
    pstep = base.ap[0][0]
    return bass.AP(tensor=base.tensor,
                   offset=base.offset + poff * pstep + foff,
                   ap=[[pstep, pcount]] + [list(d) for d in fdims])


def _dram3(dram_ap, psize, nt, tstride, inner):
    """3D dram AP: [[row, psize], [tstride, nt], [1, inner]]."""
    return bass.AP(tensor=dram_ap.tensor, offset=dram_ap.offset,
                   ap=[[inner, psize], [tstride, nt], [1, inner]])


def _build():
    nc = bacc.Bacc("TRN2", target_bir_lowering=False, debug=False,
                   num_devices=N_CORES)
    xT = nc.dram_tensor("xT", [BPC, D, L], BF16, kind="ExternalInput")
    w_qk = nc.dram_tensor("w_qk", [D, 2 * D], BF16, kind="ExternalInput")
    w_v = nc.dram_tensor("w_v", [D, D], BF16, kind="ExternalInput")
    w_o = nc.dram_tensor("w_o", [D, D], BF16, kind="ExternalInput")
    b_q = nc.dram_tensor("b_q", [D, 1], F32, kind="ExternalInput")
    b_v = nc.dram_tensor("b_v", [1, D], F32, kind="ExternalInput")
    b_o = nc.dram_tensor("b_o", [D, 1], F32, kind="ExternalInput")
    yT = nc.dram_tensor("yT", [BPC, KT, 128, 1024], BF16,
                        kind="ExternalOutput")
    kTo = nc.dram_tensor("kTo", [BPC, KT, 128, 2080], FP8,
                         kind="ExternalOutput")
    vo = nc.dram_tensor("vo", [BPC, 9, 128, H * VW], BF16,
                        kind="ExternalOutput")
    p8o = nc.dram_tensor("p8o", [BPC, 128, 96], BF16, kind="ExternalOutput")
    dno = nc.dram_tensor("dno", [BPC, 128, 96], F32, kind="ExternalOutput")

    with tile.TileContext(nc) as tc:
        _emit(nc, tc, xT, w_qk, w_v, w_o, b_q, b_v, b_o, yT, kTo, vo, p8o,
              dno)
    nc.compile()
    return nc


def _emit(nc, tc, xT, w_qk, w_v, w_o, b_q, b_v, b_o, yT, kTo, vo, p8o, dno):
    ctx = contextlib.ExitStack()
    with ctx:
        consts = ctx.enter_context(tc.tile_pool(name="consts", bufs=1))
        xpool = ctx.enter_context(tc.tile_pool(name="xpool", bufs=2))
        qpool = ctx.enter_context(tc.tile_pool(name="qpool", bufs=2))
        kpool = ctx.enter_context(tc.tile_pool(name="kpool", bufs=2))
        vpool = ctx.enter_context(tc.tile_pool(name="vpool", bufs=2))
        ptpool = ctx.enter_context(tc.tile_pool(name="ptpool", bufs=16))
        osbpool = ctx.enter_context(tc.tile_pool(name="osbpool", bufs=2))
        otpool = ctx.enter_context(tc.tile_pool(name="otpool", bufs=1))
        recpool = ctx.enter_context(tc.tile_pool(name="recpool", bufs=3))
        p8pool = ctx.enter_context(tc.tile_pool(name="p8pool", bufs=2))
        ytpool = ctx.enter_context(tc.tile_pool(name="ytpool", bufs=2))
        # PSUM: scores 2x[128,1024]=4 banks, pv 2x[128,260]=2, proj 2x=2
        bigp = ctx.enter_context(tc.tile_pool(name="bigp", bufs=2,
                                              space="PSUM"))
        pvp = ctx.enter_context(tc.tile_pool(name="pvp", bufs=2,
                                             space="PSUM"))
        projp = ctx.enter_context(tc.tile_pool(name="projp", bufs=2,
                                               space="PSUM"))

        wqk_t = consts.tile([128, KT * 2 * D], BF16, name="wqk")
        wv_t = consts.tile([128, KT * D], BF16, name="wv")
        wo_t = consts.tile([128, KT * D], BF16, name="wo")
        bq_t = consts.tile([128, KT], F32, name="bq")
        bo_t = consts.tile([128, KT], F32, name="bo")
        bv_bc = consts.tile([128, D], F32, name="bvbc")

        def wqk(k, a, n):
            return _ap(wqk_t, 0, 128, k * 2 * D + a, [[1, n]])

        def wv(k, a, n):
            return _ap(wv_t, 0, 128, k * D + a, [[1, n]])

        def wo(k, a, n):
            return _ap(wo_t, 0, 128, k * D + a, [[1, n]])

        xt = {}
        qhi = {}
        khilo = {}
        vt = {}
        osb = {}
        oTt = {}
        p8 = {}
        dnF = {}

        def alloc_elem(e):
            qhi[e] = [qpool.tile([128, 1024], FP8, tag=f"qhi{m}",
                                 name=f"qhi{e}_{m}") for m in range(KT)]
            khilo[e] = [kpool.tile([128, 2080], FP8, tag=f"kh{m}",
                                   name=f"kh{e}_{m}") for m in range(KT)]
            vt[e] = [vpool.tile([128, H * VW], BF16, tag=f"vt{j}",
                                name=f"vt{e}_{j}") for j in range(9)]
            osb[e] = osbpool.tile([128, 8 * D], BF16, tag="osb",
                                  name=f"osb{e}")
            oTt[e] = otpool.tile([128, KT * 1024], BF16, tag="oT",
                                 name=f"oT{e}")
            p8[e] = p8pool.tile([128, 96], BF16, tag="p8", name=f"p8_{e}")
            dnF[e] = p8pool.tile([128, 96], F32, tag="dn", name=f"dn{e}")

        def xtap(e, k, a, n):
            return _ap(xt[e], 0, 128, k * L + a, [[1, n]])

        def load_x(e):
            xt[e] = xpool.tile([128, KT * L], BF16, tag="xt", name=f"xt{e}")
            nc.sync.dma_start(
                out=_ap(xt[e], 0, 128, 0, [[L, KT], [1, L]]),
                in_=_dram3(xT[e], 128, KT, 128 * L, L))

        def load_x_gen(e):
            load_x(e)
            yield

        def v_unit(e, j):
            """V_aug tile [jlen, 12*65]: per head 64 V cols + ones col."""
            jlen = min(128, L - j * 128)
            nc.vector.memset(
                _ap(vt[e][j], 0, 128, DH, [[VW, H], [1, 1]]), 1.0)
            for (c0, nh) in ((0, 8), (512, 4)):
                w = nh * DH
                ps = projp.tile([128, 512], F32, tag="proj",
                                name=f"vps{e}_{j}_{c0}")
                for k in range(KT):
                    nc.tensor.matmul(ps[:jlen, :w],
                                     xtap(e, k, j * 128, jlen),
                                     wv(k, c0, w),
                                     start=(k == 0), stop=(k == KT - 1))
                    if k % 2 == 1:
                        yield
                nc.vector.tensor_tensor(
                    out=_ap(vt[e][j], 0, jlen, (c0 // DH) * VW,
                            [[VW, nh], [1, DH]]),
                    in0=_ap(ps, 0, jlen, 0, [[DH, nh], [1, DH]]),
                    in1=_ap(bv_bc, 0, jlen, c0, [[DH, nh], [1, DH]]),
                    op=ADD)
            nc.sync.dma_start(out=vo[e, j][0:jlen, :], in_=vt[e][j][:jlen, :])

        def qk_unit(e, m):
            """m 0..5: Q m-tile -> qhi (fp8, +bias). m 6..11: K m-tile ->
            khilo hi/lo planes (fp8, biasless) + straggler col + export."""
            for c in (0, 1):
                ps = projp.tile([128, 512], F32, tag="proj",
                                name=f"qkps{e}_{m}_{c}")
                for k in range(KT):
                    nc.tensor.matmul(ps[:, :],
                                     wqk(k, m * 128, 128),
                                     xtap(e, k, c * 512, 512),
                                     start=(k == 0), stop=(k == KT - 1))
                    if k % 2 == 1:
                        yield
                if m < KT:
                    nc.vector.tensor_scalar_add(
                        out=_ap(qhi[e][m], 0, 128, c * 512, [[1, 512]]),
                        in0=ps[:, :],
                        scalar1=_ap(bq_t, 0, 128, m, [[1, 1]]))
                else:
                    kh = khilo[e][m - KT]
                    nc.vector.tensor_copy(
                        out=_ap(kh, 0, 128, c * 512, [[1, 512]]),
                        in_=ps[:, :])
                    nc.vector.tensor_tensor(
                        out=_ap(kh, 0, 128, KLO + c * 512, [[1, 512]]),
                        in0=ps[:, :],
                        in1=_ap(kh, 0, 128, c * 512, [[1, 512]]), op=SUB)
                    yield
            if m >= KT:
                kh = khilo[e][m - KT]
                ps = projp.tile([128, 512], F32, tag="proj",
                                name=f"qksg{e}_{m}")
                for k in range(KT):
                    nc.tensor.matmul(ps[:, 0:1],
                                     wqk(k, m * 128, 128),
                                     xtap(e, k, 1024, 1),
                                     start=(k == 0), stop=(k == KT - 1))
                yield
                nc.vector.tensor_copy(out=_ap(kh, 0, 128, 1024, [[1, 1]]),
                                      in_=ps[:, 0:1])
                nc.vector.tensor_tensor(
                    out=_ap(kh, 0, 128, KLO + 1024, [[1, 1]]),
                    in0=ps[:, 0:1], in1=_ap(kh, 0, 128, 1024, [[1, 1]]),
                    op=SUB)
                nc.sync.dma_start(out=kTo[e, m - KT], in_=kh[:])

        def o_unit(e, m):
            yt = ytpool.tile([128, 1024], BF16, tag="yt", name=f"yt{e}_{m}")
            for c in (0, 1):
                ps = projp.tile([128, 512], F32, tag="proj",
                                name=f"ops{e}_{m}_{c}")
                for k in range(KT):
                    nc.tensor.matmul(
                        ps[:, :], wo(k, m * 128, 128),
                        _ap(oTt[e], 0, 128, k * 1024 + c * 512, [[1, 512]]),
                        start=(k == 0), stop=(k == KT - 1))
                    if k % 2 == 1:
                        yield
                nc.vector.tensor_scalar_add(
                    out=yt[:, c * 512:c * 512 + 512], in0=ps[:, :],
                    scalar1=_ap(bo_t, 0, 128, m, [[1, 1]]))
            nc.sync.dma_start(out=yT[e, m], in_=yt[:])

        class Fill:
            def __init__(self, gens):
                self.gens = list(gens)

            def pull(self, n=1):
                while n > 0 and self.gens:
                    try:
                        next(self.gens[0])
                        n -= 1
                    except StopIteration:
                        self.gens.pop(0)

            def finish(self, k):
                for gen in self.gens[:k]:
                    for _ in gen:
                        pass
                self.gens = self.gens[k:]

            def append(self, gen):
                self.gens.append(gen)

            def flush(self):
                self.finish(len(self.gens))

        # ---- software-pipelined attention: head h's score/exp loop
        # overlaps head h-1's PV accumulation so ACT never idles ----
        state = {"pend": None, "pts": None, "pv": None}

        def pv_step(qg):
            """One q-block-group of the pending head's PV: 8 consecutive
            matmuls accumulating over all key blocks. Groups sharing a
            psum bank must be consecutive - a start=True clears the whole
            bank's has_written bits on hardware."""
            e2, h2 = state["pend"]
            pts = state["pts"]
            pva, pvb = state["pv"]
            pv = pva if qg < 4 else pvb
            qb = qg % 4
            for j in range(NJ):
                nc.tensor.matmul(
                    pv[:, qb * VW:qb * VW + VW],
                    pts[j][:, qg * 128:qg * 128 + 128],
                    vt[e2][j][:, h2 * VW:h2 * VW + VW],
                    start=(j == 0), stop=(j == NJ - 1))

        def finalize_pend():
            """Normalize the pending head: rec = 1/(D8+p8); O_sb = pv*rec."""
            e2, h2 = state["pend"]
            pva, pvb = state["pv"]
            for c, pv in ((0, pva), (1, pvb)):
                nc.vector.tensor_tensor(
                    out=_ap(dnF[e2], 0, 128, h2 * 8 + c * 4, [[1, 4]]),
                    in0=_ap(pv, 0, 128, DH, [[VW, 4]]),
                    in1=_ap(p8[e2], 0, 128, h2 * 8 + c * 4, [[1, 4]]),
                    op=ADD)
                rec = recpool.tile([128, 4], F32, tag="rec",
                                   name=f"rec{e2}_{h2}_{c}")
                nc.vector.reciprocal(rec[:, :],
                                     _ap(dnF[e2], 0, 128, h2 * 8 + c * 4,
                                         [[1, 4]]))
                nc.vector.tensor_tensor(
                    out=_ap(osb[e2], 0, 128, c * 4 * D + h2 * 64,
                            [[D, 4], [1, 64]]),
                    in0=_ap(pv, 0, 128, 0, [[VW, 4], [1, 64]]),
                    in1=_ap(rec, 0, 128, 0, [[1, 4], [0, 64]]), op=MULT)
            if h2 == H - 1:
                for qb in range(8):
                    nc.sync.dma_start(
                        out=_ap(oTt[e2], 0, 128, qb * 128,
                                [[1024, KT], [1, 128]]),
                        in_=_ap(osb[e2], 0, 128, qb * D, [[1, D]]),
                        transpose=True)
                nc.sync.dma_start(out=p8o[e2], in_=p8[e2][:])
                nc.sync.dma_start(out=dno[e2], in_=dnF[e2][:])
            state["pend"] = None

        def s8_pair(e, mp):
            """Straggler-key scores + exp for head pair (2mp, 2mp+1), as a
            filler unit through the proj psum pool."""
            ps = projp.tile([128, 512], F32, tag="proj", name=f"s8_{e}_{mp}")
            for u in (0, 1):
                h = 2 * mp + u
                poff = u * 64
                kh, qh = khilo[e][mp], qhi[e][mp]
                for qb in range(8):
                    nc.tensor.matmul(
                        _ap(ps, 0, 128, u * 8 + qb, [[1, 1]]),
                        _ap(qh, poff, 64, qb * 128, [[1, 128]]),
                        _ap(kh, poff, 64, 1024, [[1, 1]]),
                        start=True, stop=True)
                yield
            nc.scalar.activation(_ap(p8[e], 0, 128, 2 * mp * 8, [[1, 16]]),
                                 _ap(ps, 0, 128, 0, [[1, 16]]),
                                 EXP, bias=0.0, scale=float(SCALE))

        def head_loop(e, h, fill):
            mq, poff = h // 2, (h % 2) * 64
            kh, qh = khilo[e][mq], qhi[e][mq]
            # scores: one DoubleRow matmul per (j, q-half):
            # S^T = Khi^T.Qhi + Klo^T.Qhi; the previous head's PV rides
            # along one q-block group per step, two steps delayed so its
            # psum-slot WAR on the normalize never blocks the queue
            pts = []
            for j in range(NJ):
                sps = bigp.tile([128, 1024], F32, tag="big",
                                name=f"sps{e}_{h}_{j}")
                for c in (0, 1):
                    nc.tensor.matmul(
                        sps[:, c * 512:c * 512 + 512],
                        _ap(kh, poff, 64, j * 128, [[KLO, 2], [1, 128]]),
                        _ap(qh, poff, 64, c * 512, [[0, 2], [1, 512]]),
                        start=True, stop=True, perf_mode=DR)
                pt = ptpool.tile([128, 1024], BF16, tag="pt",
                                 name=f"pt{e}_{h}_{j}")
                nc.scalar.activation(pt[:, :], sps[:, :], EXP,
                                     bias=0.0, scale=float(SCALE))
                pts.append(pt)
                if state["pend"] is not None and j >= 2:
                    pv_step(j - 2)
                fill.pull(2)
            if state["pend"] is not None:
                pv_step(6)
                pv_step(7)
                finalize_pend()
            pva = pvp.tile([128, 260], F32, tag="pv", name=f"pv{e}_{h}_0")
            pvb = pvp.tile([128, 260], F32, tag="pv", name=f"pv{e}_{h}_1")
            state["pend"] = (e, h)
            state["pts"] = pts
            state["pv"] = (pva, pvb)
            fill.pull(1)

        def drain_pend(fill):
            for qg in range(NJ):
                pv_step(qg)
                fill.pull(1)
            finalize_pend()

        # ---- schedule ----
        # warm the exp table during the input DMA shadow
        warm = recpool.tile([1, 4], F32, tag="warm", name="warm")
        nc.vector.memset(warm[:1, 0:1], 0.0)
        nc.scalar.activation(warm[:1, 0:1], warm[:1, 0:1], EXP,
                             bias=0.0, scale=1.0)
        nc.sync.dma_start(out=_ap(wqk_t, 0, 128, 0, [[2 * D, KT], [1, 2 * D]]),
                          in_=_dram3(w_qk[:, :], 128, KT, 128 * 2 * D, 2 * D))
        load_x(0)
        nc.sync.dma_start(out=_ap(wv_t, 0, 128, 0, [[D, KT], [1, D]]),
                          in_=_dram3(w_v[:, :], 128, KT, 128 * D, D))
        bva = b_v[:]
        nc.sync.dma_start(out=bv_bc[:], in_=bass.AP(
            tensor=bva.tensor, offset=bva.offset,
            ap=[[0, 128], list(bva.ap[1])]))
        nc.sync.dma_start(out=_ap(bq_t, 0, 128, 0, [[1, KT]]),
                          in_=_dram3(b_q[:, :], 128, KT, 128, 1))
        nc.sync.dma_start(out=_ap(bo_t, 0, 128, 0, [[1, KT]]),
                          in_=_dram3(b_o[:, :], 128, KT, 128, 1))
        nc.sync.dma_start(out=_ap(wo_t, 0, 128, 0, [[D, KT], [1, D]]),
                          in_=_dram3(w_o[:, :], 128, KT, 128 * D, D))

        def run(gen):
            for _ in gen:
                pass

        alloc_elem(0)
        run(qk_unit(0, 0))
        run(qk_unit(0, 6))
        run(s8_pair(0, 0))
        for j in range(9):
            run(v_unit(0, j))

        gens = []
        for m in range(1, KT):
            gens += [qk_unit(0, m), qk_unit(0, KT + m), s8_pair(0, m)]
        gens += [load_x_gen(1)]
        alloc_elem(1)
        gens += [v_unit(1, j) for j in range(9)]
        gens += [qk_unit(1, 0), qk_unit(1, KT), s8_pair(1, 0)]
        fill = Fill(gens)
        for h in range(H):
            if h >= 2 and h % 2 == 0:
                fill.finish(3)
            head_loop(0, h, fill)
        fill.flush()
        # element 1's attention: its scores pipeline over element 0's
        # last PV; element 0's output projection (needs ALL of O^T(0),
        # finalized inside head (1,0)) fills from head (1,1) on
        gens2 = []
        for m in range(1, KT):
            gens2 += [qk_unit(1, m), qk_unit(1, KT + m), s8_pair(1, m)]
        fill2 = Fill(gens2)
        for h in range(H):
            if h >= 2 and h % 2 == 0:
                fill2.finish(3)
            head_loop(1, h, fill2)
            if h == 1:
                for m in range(KT):
                    fill2.append(o_unit(0, m))
        drain_pend(fill2)
        fill2.flush()
        for m in range(KT):
            run(o_unit(1, m))


def _prep_inputs(query, Wqkv, bqkv, Wo, bo):
    Wp = Wqkv.reshape(D, 3, DH, H).transpose(0, 1, 3, 2).reshape(D, 3 * D)
    bp = bqkv.reshape(3, DH, H).transpose(0, 2, 1).reshape(3 * D)
    w_qk = np.ascontiguousarray(Wp[:, :2 * D]).astype(ml_dtypes.bfloat16)
    w_v = np.ascontiguousarray(Wp[:, 2 * D:]).astype(ml_dtypes.bfloat16)
    w_o = np.ascontiguousarray(Wo).astype(ml_dtypes.bfloat16)
    b_q = np.ascontiguousarray(bp[:D]).astype(np.float32).reshape(D, 1)
    b_v = np.ascontiguousarray(bp[2 * D:]).astype(np.float32).reshape(1, D)
    b_o = np.ascontiguousarray(bo).astype(np.float32).reshape(D, 1)

    in_maps = []
    for c in range(N_CORES):
        xc = query[c * BPC:(c + 1) * BPC]
        xTc = np.ascontiguousarray(xc.transpose(0, 2, 1)).astype(
            ml_dtypes.bfloat16)
        in_maps.append(dict(xT=xTc, w_qk=w_qk, w_v=w_v, w_o=w_o,
                            b_q=b_q, b_v=b_v, b_o=b_o))
    return in_maps


def kernel(query, Wqkv, bqkv, Wo, bo):
    query = np.asarray(query, dtype=np.float32)
    Wqkv = np.asarray(Wqkv, dtype=np.float32)
    bqkv = np.asarray(bqkv, dtype=np.float32)
    Wo = np.asarray(Wo, dtype=np.float32)
    bo = np.asarray(bo, dtype=np.float32)

    if "nc" not in _CACHE:
        _CACHE["nc"] = _build()
    nc = _CACHE["nc"]

    in_maps = _prep_inputs(query, Wqkv, bqkv, Wo, bo)
    res = run_bass_kernel_spmd(nc, in_maps, core_ids=list(range(N_CORES)))

    Wp = Wqkv.reshape(D, 3, DH, H).transpose(0, 1, 3, 2).reshape(D, 3 * D)
    bp = bqkv.reshape(3, DH, H).transpose(0, 2, 1).reshape(3 * D)
    out = np.empty((B, L, D), dtype=np.float32)
    for c in range(N_CORES):
        r = res.results[c]
        for e in range(BPC):
            b = c * BPC + e
            # main output: Y^T tiles [6, 128, 1024] -> Y [1024, 768]
            y = np.asarray(r["yT"][e], dtype=np.float32).reshape(
                D, 1024).T.copy()
            # rank-12 straggler-key correction: Y += (p8/D) @ (v_1024 Wo_h)
            p8v = np.asarray(r["p8o"][e], dtype=np.float32)
            dnv = np.asarray(r["dno"][e], dtype=np.float32)
            p8n = (p8v / dnv).reshape(128, H, 8).transpose(2, 0, 1).reshape(
                1024, H)
            vfull = np.asarray(r["vo"][e], dtype=np.float32).reshape(
                9 * 128, H, VW)[:, :, :DH]
            v1024 = vfull[1024]  # [H, 64]
            w8v = np.einsum("hd,hde->he", v1024, Wo.reshape(H, DH, D))
            y += p8n @ w8v
            out[b, :1024] = y
            # straggler query row: exact host attention from exported K/V
            kt8 = np.asarray(r["kTo"][e], dtype=np.float32)  # [6, 128, 2080]
            kT = (kt8[:, :, :L] + kt8[:, :, KLO:KLO + L]).reshape(D, L)
            qrow = query[b, L - 1] @ Wp[:, :D] + bp[:D]
            orow = np.empty(D, dtype=np.float32)
            for h in range(H):
                kh = kT[h * DH:(h + 1) * DH]  # [64, L]
                sh = (qrow[h * DH:(h + 1) * DH] @ kh) * SCALE
                ph = np.exp(sh - sh.max())
                vh = vfull[:L, h]
                orow[h * DH:(h + 1) * DH] = (ph @ vh) / ph.sum()
            out[b, L - 1] = orow @ Wo + bo
    return out
